# revision 16
# baseline (speedup 1.0000x reference)
"""BERT+CRF loss (torchcrf-style, reduction=sum) on 8 Trainium2 NeuronCores.

Strategy (pure data parallel, batch sharded 8 ways, 8 sequences per core):
  X is quantized to fp8-e4m3 on the host (4x less HBM traffic than f32) and
  streamed once through TensorE with DoubleRow fp8 matmuls (256-deep k-tiles)
  to produce emissions^T per sequence.  The CRF forward recurrence is
  reformulated in exp space: step matrix M_t[i,j] = expT[i,j] * E_t[j] with
  E_t = exp(em_t + b).  Triples of steps (t = 3q+2, 3q+3, 3q+4) collapse
  into one 9x9 transfer matrix via a bilinear host constant G6 [81, 81]:
      T_q[i,j] = sum_{k,l} expT[i,k] Ea[k] expT[k,l] Eb[l] expT[l,j]
  so  M_{3q+2} M_{3q+3} M_{3q+4} = T_q * diag(E_{3q+4}).
  The replicated log-space outer sum (ema[k]+emb[l]) is built with two
  accumulating indicator matmuls over a two-sequence 340-wide view and
  exponentiated in one activation; a matmul against G6 yields the pair's
  T^T [81, 2*170].  The device ships the T matrices (bf16) and the raw
  scaled emissions (bf16); the host multiplies each 9x9 chain in f64 with
  an order-preserving normalized tree reduce (O(B*170*81) work), applies
  the diag factors, and gathers the label-indexed numerator terms.
  Sequences are processed in pairs, software-pipelined so TensorE always
  has independent DoubleRow work queued; a dummy-matmul warmup stream runs
  during the DMA ramp to hold the PE's HAM clock-gate at full rate.
"""

import os
import sys

if "/opt/trn_rl_repo" not in sys.path:
    sys.path.insert(0, "/opt/trn_rl_repo")

import ml_dtypes
import numpy as np

B, S, H, L = 64, 512, 768, 9
NCORES = 8
BPC = B // NCORES          # sequences per core
NSP = BPC // 2             # sequence pairs per core
LL = L * L                 # 81
NT = 170                   # triples per sequence: steps t=2..511; t=1 on host
HC = H // 128              # 6 h-chunks of 128
NKT = 3                    # DoubleRow k-tiles (256-deep each)
MP = 16                    # DoubleRow needs >=16 weight cols per plane
NWARM = 14                 # PE warmup dummy matmuls during the DMA ramp
SCALE_W = 64.0             # W is scaled into fp8 range; exp() unscales

_CACHE = {}


def _build_bass():
    import concourse.bass as bass
    import concourse.bacc as bacc
    import concourse.mybir as mybir
    import concourse.tile as tile
    from contextlib import ExitStack

    f32 = mybir.dt.float32
    bf16 = mybir.dt.bfloat16
    f8 = mybir.dt.float8e4
    Act = mybir.ActivationFunctionType
    DR = mybir.MatmulPerfMode.DoubleRow

    nc = bacc.Bacc()

    # ---- I/O ----
    x8_d = nc.dram_tensor("x8", [BPC, 128, NKT, S, 2], f8, kind="ExternalInput")
    w8_d = nc.dram_tensor("w8", [128, HC, MP], f8, kind="ExternalInput")
    gp_d = nc.dram_tensor("Gpack", [LL, 3 * LL], bf16, kind="ExternalInput")
    cf_d = nc.dram_tensor("Cf32", [LL, 1], f32, kind="ExternalInput")

    t_out = nc.dram_tensor("t_out", [LL, BPC, NT], bf16, kind="ExternalOutput")
    em_out = nc.dram_tensor("em_out", [L, BPC, S], bf16, kind="ExternalOutput")

    with ExitStack() as ctx:
        tc = ctx.enter_context(tile.TileContext(nc))
        const = ctx.enter_context(tc.tile_pool(name="const", bufs=1))
        xpool = ctx.enter_context(tc.tile_pool(name="x", bufs=4))
        upool = ctx.enter_context(tc.tile_pool(name="u", bufs=2))
        rpool = ctx.enter_context(tc.tile_pool(name="res", bufs=1))
        ps_em = ctx.enter_context(tc.tile_pool(name="psem", bufs=2, space="PSUM"))
        ps_sm = ctx.enter_context(tc.tile_pool(name="pssm", bufs=3, space="PSUM"))

        # ---- stream X first: all four pair-DMAs dispatched up front ----
        xts = []
        for p in range(NSP):
            xt = xpool.tile([128, 2, NKT, S, 2], f8)
            nc.sync.dma_start(
                xt[:], x8_d[2 * p : 2 * p + 2].rearrange("b p t s two -> p b t s two")
            )
            xts.append(xt)

        # ---- constants (scalar queue; never blocks the X stream) ----
        gp_sb = const.tile([LL, 3 * LL], bf16)
        nc.scalar.dma_start(gp_sb[:], gp_d[:])
        w8_sb = const.tile([128, HC, MP], f8)
        nc.scalar.dma_start(w8_sb[:], w8_d[:])
        cf_sb = const.tile([LL, 1], f32)
        nc.scalar.dma_start(cf_sb[:], cf_d[:])
        g6_ap = gp_sb[:, 0:LL]
        ra_ap = gp_sb[0:L, LL : 2 * LL]
        rb_ap = gp_sb[0:L, 2 * LL : 3 * LL]
        bias81 = cf_sb[:, 0:1]

        # ---- persistent result collect tiles ----
        coll_sb = rpool.tile([LL, BPC, NT], bf16)
        emall_sb = rpool.tile([L, BPC, S], bf16)

        # ---- PE warmup: dummy matmuls on uninitialized SBUF keep the HAM
        # clock-gate at full rate through the DMA ramp; they scribble pair
        # 0's PSUM tile, which the first real matmul resets (start=True) ----
        wg_sb = const.tile([LL, LL], bf16)
        nc.vector.memset(wg_sb[:], 1.0)
        warm_ps = ps_sm.tile([LL, 2, NT], f32, tag="smt")
        for _ in range(NWARM):
            nc.tensor.matmul(
                warm_ps[:, 0, 0:LL], wg_sb[:], wg_sb[:],
                start=True, stop=True, skip_group_check=True,
            )

        emps = [None] * NSP
        usbs = [None] * NSP
        g6ps = [None] * NSP

        def stage_mm(p):
            # emissions^T: 3 DoubleRow k-tiles per sequence (moving operand
            # has the two 128-deep k-planes interleaved for the 2x fp8 rate)
            em_ps = ps_em.tile([MP, 2, S], f32, tag="em")
            for i in range(2):
                for t in range(NKT):
                    nc.tensor.matmul(
                        em_ps[:, i],
                        w8_sb[:, 2 * t : 2 * t + 2, :],
                        xts[p][:, i, t].rearrange("p s two -> p two s"),
                        start=(t == 0), stop=(t == NKT - 1),
                        perf_mode=DR,
                    )
            emps[p] = em_ps

        def stage_cast(p):
            # scaled emissions to SBUF (bf16): feeds the replicate matmuls,
            # and shipped raw to the host (numerator gather + exp columns)
            nc.vector.tensor_copy(emall_sb[:, 2 * p : 2 * p + 2], emps[p][0:L])

        def stage_rep(p):
            # rep[(k,l), (i,q)] = ema[k, i, 3q+2] + emb[l, i, 3q+3] via two
            # accumulating indicator matmuls over the 340-wide pair view
            emv = emall_sb[:, 2 * p : 2 * p + 2]
            ap0 = emall_sb[:].ap[0]
            ea_ap = bass.AP(
                emall_sb.tensor, emv.offset + 2, [[ap0[0], L], [S, 2], [3, NT]]
            )
            eb_ap = bass.AP(
                emall_sb.tensor, emv.offset + 3, [[ap0[0], L], [S, 2], [3, NT]]
            )
            rep_ps = ps_sm.tile([LL, 2, NT], f32, tag="smt")
            nc.tensor.matmul(rep_ps[:], ra_ap, ea_ap, start=True, stop=False)
            nc.tensor.matmul(rep_ps[:], rb_ap, eb_ap, start=False, stop=True)
            return rep_ps

        def stage_uexp(p, rep_ps):
            u_sb = upool.tile([LL, 2, NT], bf16)
            nc.scalar.activation(
                u_sb[:], rep_ps[:], Act.Exp, bias=bias81, scale=1.0 / SCALE_W
            )
            usbs[p] = u_sb

        def stage_g6(p):
            # T^T [(i,j), (i,q)] = G6^T @ u  (pair of triple-transfer matrices)
            t_ps = ps_sm.tile([LL, 2, NT], f32, tag="smt")
            nc.tensor.matmul(t_ps[:], g6_ap, usbs[p][:], start=True, stop=True)
            g6ps[p] = t_ps

        def stage_coll(p):
            nc.scalar.copy(coll_sb[:, 2 * p : 2 * p + 2], g6ps[p][:])
            emps[p] = usbs[p] = g6ps[p] = None

        stage_mm(0)
        stage_cast(0)
        stage_mm(1)
        stage_cast(1)
        stage_uexp(0, stage_rep(0))
        for p in range(2, NSP):
            stage_mm(p)
            stage_cast(p)
            stage_uexp(p - 1, stage_rep(p - 1))
            stage_g6(p - 2)
            stage_coll(p - 2)
        stage_uexp(NSP - 1, stage_rep(NSP - 1))
        stage_g6(NSP - 2)
        stage_coll(NSP - 2)
        # first-half outputs overlap with the last pair's tail
        nc.sync.dma_start(t_out[:, 0:4], coll_sb[:, 0:4])
        nc.sync.dma_start(em_out[:, 0:4], emall_sb[:, 0:4])
        stage_g6(NSP - 1)
        stage_coll(NSP - 1)

        nc.sync.dma_start(t_out[:, 4:BPC], coll_sb[:, 4:BPC])
        nc.sync.dma_start(em_out[:, 4:BPC], emall_sb[:, 4:BPC])

    if not nc.is_finalized():
        nc.finalize()
    return nc


def _get_nc():
    if "nc" not in _CACHE:
        _CACHE["nc"] = _build_bass()
    return _CACHE["nc"]


def _host_consts(trans, bb):
    expT = np.exp(trans.astype(np.float64))                      # [9,9] f64
    r = np.arange(LL)
    c = np.arange(LL)
    k = r // L
    l = r % L
    i = c // L
    j = c % L
    gpack = np.zeros((LL, 3 * LL), dtype=ml_dtypes.bfloat16)
    # G6[(k,l), (i,j)] = expT[i,k] * expT[k,l] * expT[l,j]
    gpack[:, 0:LL] = (
        expT[i[None, :], k[:, None]]
        * expT[k[:, None], l[:, None]]
        * expT[l[:, None], j[None, :]]
    ).astype(ml_dtypes.bfloat16)
    gpack[0:L, LL : 2 * LL] = k[None, :] == np.arange(L)[:, None]
    gpack[0:L, 2 * LL : 3 * LL] = l[None, :] == np.arange(L)[:, None]
    b64 = bb.astype(np.float64)
    cf = (b64[k] + b64[l]).astype(np.float32).reshape(LL, 1)
    return expT, gpack, cf


def _numpy_reference(hs, mask, labels, W, bb, st, en, tr):
    # general fallback (only used when attention_mask is not all ones)
    em = hs.astype(np.float64) @ W.astype(np.float64) + bb.astype(np.float64)
    maskb = mask.astype(bool)
    maskf = mask.astype(np.float64)
    em_tag = np.take_along_axis(em, labels[..., None], axis=-1)[..., 0]
    num = st.astype(np.float64)[labels[:, 0]] + em_tag[:, 0]
    trs = tr.astype(np.float64)[labels[:, :-1], labels[:, 1:]]
    num = num + np.sum((trs + em_tag[:, 1:]) * maskf[:, 1:], axis=1)
    last = mask.sum(axis=1).astype(np.int64) - 1
    num = num + en.astype(np.float64)[labels[np.arange(len(labels)), last]]
    alpha = st.astype(np.float64)[None, :] + em[:, 0]
    for t in range(1, em.shape[1]):
        x = alpha[:, :, None] + tr.astype(np.float64)[None, :, :] + em[:, t][:, None, :]
        m = x.max(axis=1, keepdims=True)
        nxt = np.log(np.exp(x - m).sum(axis=1)) + m[:, 0, :]
        alpha = np.where(maskb[:, t][:, None], nxt, alpha)
    x = alpha + en.astype(np.float64)[None, :]
    m = x.max(axis=1, keepdims=True)
    denom = np.log(np.exp(x - m).sum(axis=1)) + m[:, 0]
    return np.asarray((denom - num).sum(), dtype=np.float32)


def _run_device(nc, in_maps):
    if os.environ.get("KERNEL_SIM"):
        from concourse.bass_interp import MultiCoreSim

        sim = MultiCoreSim(nc, len(in_maps))
        for t, m in enumerate(in_maps):
            for k2, v in m.items():
                sim.cores[t].tensor(k2)[:] = v
        sim.simulate()
        outs = []
        for t in range(len(in_maps)):
            outs.append(
                {
                    name: np.array(sim.cores[t].tensor(name))
                    for name in ("t_out", "em_out")
                }
            )

        class _R:
            results = outs
            exec_time_ns = None

        return _R()
    from concourse import bass_utils

    return bass_utils.run_bass_kernel_spmd(nc, in_maps, list(range(len(in_maps))))


def kernel(**inputs):
    hs = np.asarray(inputs["hidden_states"], dtype=np.float32)
    mask = np.asarray(inputs["attention_mask"])
    labels = np.asarray(inputs["labels"]).astype(np.int64)
    W = np.asarray(inputs["W"], dtype=np.float32)
    bb = np.asarray(inputs["b"], dtype=np.float32)
    st = np.asarray(inputs["start_trans"], dtype=np.float32)
    en = np.asarray(inputs["end_trans"], dtype=np.float32)
    tr = np.asarray(inputs["trans"], dtype=np.float32)

    if not np.all(mask == 1):
        return _numpy_reference(hs, mask, labels, W, bb, st, en, tr)

    expT64, gpack, cf = _host_consts(tr, bb)

    # X -> fp8 e4m3 in [B, 128, HC, S] layout (h = 128*c + p)
    xq = np.clip(hs, -224.0, 224.0).astype(ml_dtypes.float8_e4m3)   # [B, S, H]
    x8 = np.ascontiguousarray(
        xq.transpose(0, 2, 1)
        .reshape(B, NKT, 2, 128, S)
        .transpose(0, 3, 1, 4, 2)
    )                                                               # [B, 128, NKT, S, 2]
    wpad = np.zeros((H, MP), dtype=np.float32)
    wpad[:, :L] = W * SCALE_W
    w8 = np.ascontiguousarray(
        np.clip(wpad, -224.0, 224.0)
        .astype(ml_dtypes.float8_e4m3)
        .reshape(HC, 128, MP)
        .transpose(1, 0, 2)
    )                                                               # [128, HC, MP]

    nc = _get_nc()
    in_maps = []
    for k in range(NCORES):
        sl = slice(k * BPC, (k + 1) * BPC)
        in_maps.append(
            {"x8": x8[sl], "w8": w8, "Gpack": gpack, "Cf32": cf}
        )
    res = _run_device(nc, in_maps)
    _CACHE["last_results"] = res

    # ---- host combine (f64, O(B * NT * 81)) ----
    st64 = st.astype(np.float64)
    en64 = en.astype(np.float64)
    b64 = bb.astype(np.float64)
    e_en = np.exp(en64)
    e_st = np.exp(st64)
    total = 0.0
    for k in range(NCORES):
        r = res.results[k]
        em = r["em_out"].astype(np.float64) / SCALE_W       # [9, b, t] (no bias)
        Tm = (
            r["t_out"].astype(np.float64)
            .reshape(L, L, BPC, NT)
            .transpose(2, 3, 0, 1)
        )                                                   # [b, q, i, j]
        # diag factors exp(em + b) at t = 3q+4
        D = np.exp(em[:, :, 4::3] + b64[:, None, None]).transpose(1, 2, 0)
        M = Tm * D[:, :, None, :]
        logacc = np.zeros(BPC)
        while M.shape[1] > 1:
            n = M.shape[1]
            half = n // 2
            P = M[:, 0 : 2 * half : 2] @ M[:, 1 : 2 * half : 2]
            if n % 2:
                P = np.concatenate([P, M[:, 2 * half :]], axis=1)
            m = P.max(axis=(2, 3), keepdims=True)
            P /= m
            logacc += np.log(m[:, :, 0, 0]).sum(axis=1)
            M = P
        v0 = e_st[:, None] * np.exp(em[:, :, 0] + b64[:, None])     # [9, b]
        v1 = (v0.T @ expT64) * np.exp(em[:, :, 1] + b64[:, None]).T
        v = np.einsum("bj,bjk->bk", v1, M[:, 0])
        denom = np.log(v @ e_en) + logacc
        total += float(denom.sum())
        lb = labels[k * BPC : (k + 1) * BPC]                # [b, t]
        em_tag = np.take_along_axis(em.transpose(1, 2, 0), lb[:, :, None], axis=2)
        total -= float(em_tag.sum())
        total -= float(
            st64[lb[:, 0]].sum()
            + en64[lb[:, -1]].sum()
            + tr.astype(np.float64)[lb[:, :-1], lb[:, 1:]].sum()
            + bb.astype(np.float64)[lb].sum()
        )
    return np.asarray(total, dtype=np.float32)


# revision 19
# speedup vs baseline: 1.1957x; 1.1957x over previous
"""BERT+CRF loss (torchcrf-style, reduction=sum) on 8 Trainium2 NeuronCores.

Strategy (pure data parallel, batch sharded 8 ways, 8 sequences per core):
  X is quantized to fp8-e4m3 on the host (4x less HBM traffic than f32) and
  streamed once through TensorE with DoubleRow fp8 matmuls (256-deep k-tiles)
  to produce emissions^T per sequence.  The CRF forward recurrence is
  reformulated in exp space: step matrix M_t[i,j] = expT[i,j] * E_t[j] with
  E_t = exp(em_t + b).  Triples of steps (t = 3q+2, 3q+3, 3q+4) collapse
  into one 9x9 transfer matrix via a bilinear host constant G6 [81, 81]:
      T_q[i,j] = sum_{k,l} expT[i,k] Ea[k] expT[k,l] Eb[l] expT[l,j]
  so  M_{3q+2} M_{3q+3} M_{3q+4} = T_q * diag(E_{3q+4}).
  The replicated log-space outer sum (ema[k]+emb[l]) is built with two
  accumulating indicator matmuls over a two-sequence 340-wide view and
  exponentiated in one activation; a matmul against G6 yields the pair's
  T^T [81, 2*170].  The device ships the T matrices (bf16) and the raw
  scaled emissions (bf16); the host multiplies each 9x9 chain in f64 with
  an order-preserving normalized tree reduce (O(B*170*81) work), applies
  the diag factors, and gathers the label-indexed numerator terms.
  Sequences are processed in pairs, software-pipelined so TensorE always
  has independent DoubleRow work queued; a dummy-matmul warmup stream runs
  during the DMA ramp to hold the PE's HAM clock-gate at full rate.
"""

import os
import sys

if "/opt/trn_rl_repo" not in sys.path:
    sys.path.insert(0, "/opt/trn_rl_repo")

import ml_dtypes
import numpy as np

B, S, H, L = 64, 512, 768, 9
NCORES = 8
BPC = B // NCORES          # sequences per core
NSP = BPC // 2             # sequence pairs per core
LL = L * L                 # 81
NT = 170                   # triples per sequence: steps t=2..511; t=1 on host
HC = H // 128              # 6 h-chunks of 128
NKT = 3                    # DoubleRow k-tiles (256-deep each)
MP = 16                    # DoubleRow needs >=16 weight cols per plane
NWARM = 16                 # PE warmup dummy matmuls during the DMA ramp
NWCOL = 2 * NT             # warmup matmul moving width
SCALE_W = 64.0             # W is scaled into fp8 range; exp() unscales

_CACHE = {}


def _build_bass():
    import concourse.bass as bass
    import concourse.bacc as bacc
    import concourse.mybir as mybir
    import concourse.tile as tile
    from concourse.tile_rust import add_dep_helper
    from contextlib import ExitStack

    f32 = mybir.dt.float32
    bf16 = mybir.dt.bfloat16
    f8 = mybir.dt.float8e4
    Act = mybir.ActivationFunctionType
    DR = mybir.MatmulPerfMode.DoubleRow

    nc = bacc.Bacc()

    # ---- I/O ----
    x8_d = nc.dram_tensor("x8", [BPC, 128, NKT, S, 2], f8, kind="ExternalInput")
    w8_d = nc.dram_tensor("w8", [128, HC, MP], f8, kind="ExternalInput")
    gp_d = nc.dram_tensor("Gpack", [LL, 3 * LL], bf16, kind="ExternalInput")
    cf_d = nc.dram_tensor("Cf32", [LL, 1], f32, kind="ExternalInput")

    t_out = nc.dram_tensor("t_out", [LL, BPC, NT], bf16, kind="ExternalOutput")
    em_out = nc.dram_tensor("em_out", [L, BPC, S], bf16, kind="ExternalOutput")

    with ExitStack() as ctx:
        tc = ctx.enter_context(tile.TileContext(nc))
        const = ctx.enter_context(tc.tile_pool(name="const", bufs=1))
        xpool = ctx.enter_context(tc.tile_pool(name="x", bufs=4))
        upool = ctx.enter_context(tc.tile_pool(name="u", bufs=2))
        rpool = ctx.enter_context(tc.tile_pool(name="res", bufs=1))
        ps_em = ctx.enter_context(tc.tile_pool(name="psem", bufs=2, space="PSUM"))
        ps_sm = ctx.enter_context(tc.tile_pool(name="pssm", bufs=3, space="PSUM"))

        # ---- weights first (tiny, feeds the first matmul), then the X
        # stream: all four pair-DMAs dispatched up front on the sync queue ----
        w8_sb = const.tile([128, HC, MP], f8)
        nc.sync.dma_start(w8_sb[:], w8_d[:])
        xts = []
        for p in range(NSP):
            xt = xpool.tile([128, 2, NKT, S, 2], f8)
            nc.sync.dma_start(
                xt[:], x8_d[2 * p : 2 * p + 2].rearrange("b p t s two -> p b t s two")
            )
            xts.append(xt)

        # ---- remaining constants (scalar queue) ----
        gp_sb = const.tile([LL, 3 * LL], bf16)
        nc.scalar.dma_start(gp_sb[:], gp_d[:])
        cf_sb = const.tile([LL, 1], f32)
        nc.scalar.dma_start(cf_sb[:], cf_d[:])
        g6_ap = gp_sb[:, 0:LL]
        ra_ap = gp_sb[0:L, LL : 2 * LL]
        rb_ap = gp_sb[0:L, 2 * LL : 3 * LL]
        bias81 = cf_sb[:, 0:1]

        # ---- persistent result collect tiles ----
        coll_sb = rpool.tile([LL, BPC, NT], bf16)
        emall_sb = rpool.tile([L, BPC, S], bf16)

        emps = [None] * NSP
        usbs = [None] * NSP
        g6ps = [None] * NSP
        last_mm = [None]

        def tmm(*args, **kw):
            inst = nc.tensor.matmul(*args, **kw).ins
            if last_mm[0] is not None:
                add_dep_helper(inst, last_mm[0], sync=False, reason="pin tensor order")
            last_mm[0] = inst
            return inst

        # ---- PE warmup: dummy matmuls on a memset tile span the DMA ramp,
        # holding the HAM clock-gate at full rate until real work arrives ----
        wg_sb = const.tile([LL, NWCOL], bf16)
        nc.vector.memset(wg_sb[:], 1.0)
        warm_ps = ps_sm.tile([LL, 2, NT], f32, tag="smt")
        for _ in range(NWARM):
            tmm(
                warm_ps[:].rearrange("p a b -> p (a b)"), wg_sb[:, 0:LL], wg_sb[:],
                start=True, stop=True, skip_group_check=True,
            )

        def stage_mm(p):
            # emissions^T: 3 DoubleRow k-tiles per sequence (moving operand
            # has the two 128-deep k-planes interleaved for the 2x fp8 rate)
            em_ps = ps_em.tile([MP, 2, S], f32, tag="em")
            for i in range(2):
                for t in range(NKT):
                    tmm(
                        em_ps[:, i],
                        w8_sb[:, 2 * t : 2 * t + 2, :],
                        xts[p][:, i, t].rearrange("p s two -> p two s"),
                        start=(t == 0), stop=(t == NKT - 1),
                        perf_mode=DR,
                    )
            emps[p] = em_ps

        def stage_cast(p):
            # scaled emissions to SBUF (bf16): feeds the replicate matmuls,
            # and shipped raw to the host (numerator gather + exp columns)
            nc.vector.tensor_copy(emall_sb[:, 2 * p : 2 * p + 2], emps[p][0:L])

        def stage_rep(p):
            # rep[(k,l), (i,q)] = ema[k, i, 3q+2] + emb[l, i, 3q+3] via two
            # accumulating indicator matmuls over the 340-wide pair view
            emv = emall_sb[:, 2 * p : 2 * p + 2]
            ap0 = emall_sb[:].ap[0]
            ea_ap = bass.AP(
                emall_sb.tensor, emv.offset + 2, [[ap0[0], L], [S, 2], [3, NT]]
            )
            eb_ap = bass.AP(
                emall_sb.tensor, emv.offset + 3, [[ap0[0], L], [S, 2], [3, NT]]
            )
            rep_ps = ps_sm.tile([LL, 2, NT], f32, tag="smt")
            tmm(rep_ps[:], ra_ap, ea_ap, start=True, stop=False)
            tmm(rep_ps[:], rb_ap, eb_ap, start=False, stop=True)
            return rep_ps

        def stage_uexp(p, rep_ps):
            u_sb = upool.tile([LL, 2, NT], bf16)
            nc.scalar.activation(
                u_sb[:], rep_ps[:], Act.Exp, bias=bias81, scale=1.0 / SCALE_W
            )
            usbs[p] = u_sb

        def stage_g6(p):
            # T^T [(i,j), (i,q)] = G6^T @ u  (pair of triple-transfer matrices)
            t_ps = ps_sm.tile([LL, 2, NT], f32, tag="smt")
            tmm(t_ps[:], g6_ap, usbs[p][:], start=True, stop=True)
            g6ps[p] = t_ps

        def stage_coll(p):
            nc.scalar.copy(coll_sb[:, 2 * p : 2 * p + 2], g6ps[p][:])
            emps[p] = usbs[p] = g6ps[p] = None

        stage_mm(0)
        stage_cast(0)
        stage_mm(1)
        stage_cast(1)
        stage_uexp(0, stage_rep(0))
        for p in range(2, NSP):
            stage_mm(p)
            stage_cast(p)
            stage_uexp(p - 1, stage_rep(p - 1))
            stage_g6(p - 2)
            stage_coll(p - 2)
        stage_uexp(NSP - 1, stage_rep(NSP - 1))
        stage_g6(NSP - 2)
        stage_coll(NSP - 2)
        # first-half outputs overlap with the last pair's tail
        nc.sync.dma_start(t_out[:, 0:4], coll_sb[:, 0:4])
        nc.sync.dma_start(em_out[:, 0:4], emall_sb[:, 0:4])
        stage_g6(NSP - 1)
        stage_coll(NSP - 1)

        nc.sync.dma_start(t_out[:, 4:BPC], coll_sb[:, 4:BPC])
        nc.sync.dma_start(em_out[:, 4:BPC], emall_sb[:, 4:BPC])

    if not nc.is_finalized():
        nc.finalize()
    return nc


def _get_nc():
    if "nc" not in _CACHE:
        _CACHE["nc"] = _build_bass()
    return _CACHE["nc"]


def _host_consts(trans, bb):
    expT = np.exp(trans.astype(np.float64))                      # [9,9] f64
    r = np.arange(LL)
    c = np.arange(LL)
    k = r // L
    l = r % L
    i = c // L
    j = c % L
    gpack = np.zeros((LL, 3 * LL), dtype=ml_dtypes.bfloat16)
    # G6[(k,l), (i,j)] = expT[i,k] * expT[k,l] * expT[l,j]
    gpack[:, 0:LL] = (
        expT[i[None, :], k[:, None]]
        * expT[k[:, None], l[:, None]]
        * expT[l[:, None], j[None, :]]
    ).astype(ml_dtypes.bfloat16)
    gpack[0:L, LL : 2 * LL] = k[None, :] == np.arange(L)[:, None]
    gpack[0:L, 2 * LL : 3 * LL] = l[None, :] == np.arange(L)[:, None]
    b64 = bb.astype(np.float64)
    cf = (b64[k] + b64[l]).astype(np.float32).reshape(LL, 1)
    return expT, gpack, cf


def _numpy_reference(hs, mask, labels, W, bb, st, en, tr):
    # general fallback (only used when attention_mask is not all ones)
    em = hs.astype(np.float64) @ W.astype(np.float64) + bb.astype(np.float64)
    maskb = mask.astype(bool)
    maskf = mask.astype(np.float64)
    em_tag = np.take_along_axis(em, labels[..., None], axis=-1)[..., 0]
    num = st.astype(np.float64)[labels[:, 0]] + em_tag[:, 0]
    trs = tr.astype(np.float64)[labels[:, :-1], labels[:, 1:]]
    num = num + np.sum((trs + em_tag[:, 1:]) * maskf[:, 1:], axis=1)
    last = mask.sum(axis=1).astype(np.int64) - 1
    num = num + en.astype(np.float64)[labels[np.arange(len(labels)), last]]
    alpha = st.astype(np.float64)[None, :] + em[:, 0]
    for t in range(1, em.shape[1]):
        x = alpha[:, :, None] + tr.astype(np.float64)[None, :, :] + em[:, t][:, None, :]
        m = x.max(axis=1, keepdims=True)
        nxt = np.log(np.exp(x - m).sum(axis=1)) + m[:, 0, :]
        alpha = np.where(maskb[:, t][:, None], nxt, alpha)
    x = alpha + en.astype(np.float64)[None, :]
    m = x.max(axis=1, keepdims=True)
    denom = np.log(np.exp(x - m).sum(axis=1)) + m[:, 0]
    return np.asarray((denom - num).sum(), dtype=np.float32)


def _run_device(nc, in_maps):
    if os.environ.get("KERNEL_SIM"):
        from concourse.bass_interp import MultiCoreSim

        sim = MultiCoreSim(nc, len(in_maps))
        for t, m in enumerate(in_maps):
            for k2, v in m.items():
                sim.cores[t].tensor(k2)[:] = v
        sim.simulate()
        outs = []
        for t in range(len(in_maps)):
            outs.append(
                {
                    name: np.array(sim.cores[t].tensor(name))
                    for name in ("t_out", "em_out")
                }
            )

        class _R:
            results = outs
            exec_time_ns = None

        return _R()
    from concourse import bass_utils

    return bass_utils.run_bass_kernel_spmd(nc, in_maps, list(range(len(in_maps))))


def kernel(**inputs):
    hs = np.asarray(inputs["hidden_states"], dtype=np.float32)
    mask = np.asarray(inputs["attention_mask"])
    labels = np.asarray(inputs["labels"]).astype(np.int64)
    W = np.asarray(inputs["W"], dtype=np.float32)
    bb = np.asarray(inputs["b"], dtype=np.float32)
    st = np.asarray(inputs["start_trans"], dtype=np.float32)
    en = np.asarray(inputs["end_trans"], dtype=np.float32)
    tr = np.asarray(inputs["trans"], dtype=np.float32)

    if not np.all(mask == 1):
        return _numpy_reference(hs, mask, labels, W, bb, st, en, tr)

    expT64, gpack, cf = _host_consts(tr, bb)

    # X -> fp8 e4m3 in [B, 128, HC, S] layout (h = 128*c + p)
    xq = np.clip(hs, -224.0, 224.0).astype(ml_dtypes.float8_e4m3)   # [B, S, H]
    x8 = np.ascontiguousarray(
        xq.transpose(0, 2, 1)
        .reshape(B, NKT, 2, 128, S)
        .transpose(0, 3, 1, 4, 2)
    )                                                               # [B, 128, NKT, S, 2]
    wpad = np.zeros((H, MP), dtype=np.float32)
    wpad[:, :L] = W * SCALE_W
    w8 = np.ascontiguousarray(
        np.clip(wpad, -224.0, 224.0)
        .astype(ml_dtypes.float8_e4m3)
        .reshape(HC, 128, MP)
        .transpose(1, 0, 2)
    )                                                               # [128, HC, MP]

    nc = _get_nc()
    in_maps = []
    for k in range(NCORES):
        sl = slice(k * BPC, (k + 1) * BPC)
        in_maps.append(
            {"x8": x8[sl], "w8": w8, "Gpack": gpack, "Cf32": cf}
        )
    res = _run_device(nc, in_maps)
    _CACHE["last_results"] = res

    # ---- host combine (f64, O(B * NT * 81)) ----
    st64 = st.astype(np.float64)
    en64 = en.astype(np.float64)
    b64 = bb.astype(np.float64)
    e_en = np.exp(en64)
    e_st = np.exp(st64)
    total = 0.0
    for k in range(NCORES):
        r = res.results[k]
        em = r["em_out"].astype(np.float64) / SCALE_W       # [9, b, t] (no bias)
        Tm = (
            r["t_out"].astype(np.float64)
            .reshape(L, L, BPC, NT)
            .transpose(2, 3, 0, 1)
        )                                                   # [b, q, i, j]
        # diag factors exp(em + b) at t = 3q+4
        D = np.exp(em[:, :, 4::3] + b64[:, None, None]).transpose(1, 2, 0)
        M = Tm * D[:, :, None, :]
        logacc = np.zeros(BPC)
        while M.shape[1] > 1:
            n = M.shape[1]
            half = n // 2
            P = M[:, 0 : 2 * half : 2] @ M[:, 1 : 2 * half : 2]
            if n % 2:
                P = np.concatenate([P, M[:, 2 * half :]], axis=1)
            m = P.max(axis=(2, 3), keepdims=True)
            P /= m
            logacc += np.log(m[:, :, 0, 0]).sum(axis=1)
            M = P
        v0 = e_st[:, None] * np.exp(em[:, :, 0] + b64[:, None])     # [9, b]
        v1 = (v0.T @ expT64) * np.exp(em[:, :, 1] + b64[:, None]).T
        v = np.einsum("bj,bjk->bk", v1, M[:, 0])
        denom = np.log(v @ e_en) + logacc
        total += float(denom.sum())
        lb = labels[k * BPC : (k + 1) * BPC]                # [b, t]
        em_tag = np.take_along_axis(em.transpose(1, 2, 0), lb[:, :, None], axis=2)
        total -= float(em_tag.sum())
        total -= float(
            st64[lb[:, 0]].sum()
            + en64[lb[:, -1]].sum()
            + tr.astype(np.float64)[lb[:, :-1], lb[:, 1:]].sum()
            + bb.astype(np.float64)[lb].sum()
        )
    return np.asarray(total, dtype=np.float32)


# revision 20
# speedup vs baseline: 1.3224x; 1.1060x over previous
"""BERT+CRF loss (torchcrf-style, reduction=sum) on 8 Trainium2 NeuronCores.

Strategy (pure data parallel, batch sharded 8 ways, 8 sequences per core):
  X is quantized to fp8-e4m3 on the host (4x less HBM traffic than f32) and
  streamed once through TensorE with DoubleRow fp8 matmuls (256-deep k-tiles)
  to produce emissions^T per sequence.  The CRF forward recurrence is
  reformulated in exp space: step matrix M_t[i,j] = expT[i,j] * E_t[j] with
  E_t = exp(em_t + b).  Triples of steps (t = 3q+2, 3q+3, 3q+4) collapse
  into one 9x9 transfer matrix via a bilinear host constant G6 [81, 81]:
      T_q[i,j] = sum_{k,l} expT[i,k] Ea[k] expT[k,l] Eb[l] expT[l,j]
  so  M_{3q+2} M_{3q+3} M_{3q+4} = T_q * diag(E_{3q+4}).
  The replicated log-space outer sum (ema[k]+emb[l]) is built with two
  accumulating indicator matmuls over a two-sequence 340-wide view and
  exponentiated in one activation; a matmul against G6 yields the pair's
  T^T [81, 2*170].  The device ships the T matrices (bf16) and the raw
  scaled emissions (bf16); the host multiplies each 9x9 chain in f64 with
  an order-preserving normalized tree reduce (O(B*170*81) work), applies
  the diag factors, and gathers the label-indexed numerator terms.
  Sequences are processed in pairs, software-pipelined so TensorE always
  has independent DoubleRow work queued; a dummy-matmul warmup stream runs
  during the DMA ramp to hold the PE's HAM clock-gate at full rate.
"""

import os
import sys

if "/opt/trn_rl_repo" not in sys.path:
    sys.path.insert(0, "/opt/trn_rl_repo")

import ml_dtypes
import numpy as np

B, S, H, L = 64, 512, 768, 9
NCORES = 8
BPC = B // NCORES          # sequences per core
NSP = BPC // 2             # sequence pairs per core
LL = L * L                 # 81
NT = 170                   # triples per sequence: steps t=2..511; t=1 on host
HC = H // 128              # 6 h-chunks of 128
NKT = 3                    # DoubleRow k-tiles (256-deep each)
MP = 16                    # DoubleRow needs >=16 weight cols per plane
NWARM = 5                  # PE warmup dummy matmuls during the DMA ramp
NWCOL = 2 * NT             # warmup matmul moving width
SCALE_W = 64.0             # W is scaled into fp8 range; exp() unscales

_CACHE = {}


def _build_bass():
    import concourse.bass as bass
    import concourse.bacc as bacc
    import concourse.mybir as mybir
    import concourse.tile as tile
    from concourse.tile_rust import add_dep_helper
    from contextlib import ExitStack

    f32 = mybir.dt.float32
    bf16 = mybir.dt.bfloat16
    f8 = mybir.dt.float8e4
    Act = mybir.ActivationFunctionType
    DR = mybir.MatmulPerfMode.DoubleRow

    nc = bacc.Bacc()

    # ---- I/O ----
    x8_d = nc.dram_tensor("x8", [BPC, 128, NKT, S, 2], f8, kind="ExternalInput")
    w8_d = nc.dram_tensor("w8", [128, HC, MP], f8, kind="ExternalInput")
    gp_d = nc.dram_tensor("Gpack", [LL, 3 * LL], bf16, kind="ExternalInput")
    cf_d = nc.dram_tensor("Cf32", [LL, 1], f32, kind="ExternalInput")

    t_out = nc.dram_tensor("t_out", [LL, BPC, NT], bf16, kind="ExternalOutput")
    em_out = nc.dram_tensor("em_out", [L, BPC, S], bf16, kind="ExternalOutput")

    with ExitStack() as ctx:
        tc = ctx.enter_context(tile.TileContext(nc))
        const = ctx.enter_context(tc.tile_pool(name="const", bufs=1))
        xpool = ctx.enter_context(tc.tile_pool(name="x", bufs=8))
        upool = ctx.enter_context(tc.tile_pool(name="u", bufs=2))
        rpool = ctx.enter_context(tc.tile_pool(name="res", bufs=1))
        ps_em = ctx.enter_context(tc.tile_pool(name="psem", bufs=3, space="PSUM"))
        ps_sm = ctx.enter_context(tc.tile_pool(name="pssm", bufs=2, space="PSUM"))

        # ---- weights first (tiny, feeds the first matmul), then the X
        # stream: all four pair-DMAs dispatched up front on the sync queue ----
        w8_sb = const.tile([128, HC, MP], f8)
        nc.sync.dma_start(w8_sb[:], w8_d[:])
        xts = []
        for b in range(BPC):
            xt = xpool.tile([128, NKT, S, 2], f8)
            nc.sync.dma_start(xt[:], x8_d[b])
            xts.append(xt)

        # ---- remaining constants (scalar queue) ----
        gp_sb = const.tile([LL, 3 * LL], bf16)
        nc.scalar.dma_start(gp_sb[:], gp_d[:])
        cf_sb = const.tile([LL, 1], f32)
        nc.scalar.dma_start(cf_sb[:], cf_d[:])
        g6_ap = gp_sb[:, 0:LL]
        ra_ap = gp_sb[0:L, LL : 2 * LL]
        rb_ap = gp_sb[0:L, 2 * LL : 3 * LL]
        bias81 = cf_sb[:, 0:1]

        # ---- persistent result collect tiles ----
        coll_sb = rpool.tile([LL, BPC, NT], bf16)
        emall_sb = rpool.tile([L, BPC, S], bf16)

        emps = [None] * NSP
        usbs = [None] * NSP
        g6ps = [None] * NSP
        last_mm = [None]

        def tmm(*args, **kw):
            inst = nc.tensor.matmul(*args, **kw).ins
            if last_mm[0] is not None:
                add_dep_helper(inst, last_mm[0], sync=False, reason="pin tensor order")
            last_mm[0] = inst
            return inst

        # ---- PE warmup: dummy matmuls on a memset tile span the DMA ramp,
        # holding the HAM clock-gate at full rate until real work arrives ----
        wg_sb = const.tile([LL, NWCOL], bf16)
        nc.vector.memset(wg_sb[:], 1.0)
        warm_ps = ps_sm.tile([LL, 2, NT], f32, tag="smt")
        for _ in range(NWARM):
            tmm(
                warm_ps[:].rearrange("p a b -> p (a b)"), wg_sb[:, 0:LL], wg_sb[:],
                start=True, stop=True, skip_group_check=True,
            )

        def stage_mm(p):
            # emissions^T: 3 DoubleRow k-tiles per sequence (moving operand
            # has the two 128-deep k-planes interleaved for the 2x fp8 rate)
            em_ps = ps_em.tile([MP, 2, S], f32, tag="em")
            for i in range(2):
                for t in range(NKT):
                    tmm(
                        em_ps[:, i],
                        w8_sb[:, 2 * t : 2 * t + 2, :],
                        xts[2 * p + i][:, t].rearrange("p s two -> p two s"),
                        start=(t == 0), stop=(t == NKT - 1),
                        perf_mode=DR,
                    )
            emps[p] = em_ps

        def stage_cast(p):
            # scaled emissions to SBUF (bf16): feeds the replicate matmuls,
            # and shipped raw to the host (numerator gather + exp columns)
            nc.vector.tensor_copy(emall_sb[:, 2 * p : 2 * p + 2], emps[p][0:L])

        def stage_rep(p):
            # rep[(k,l), (i,q)] = ema[k, i, 3q+2] + emb[l, i, 3q+3] via two
            # accumulating indicator matmuls over the 340-wide pair view
            emv = emall_sb[:, 2 * p : 2 * p + 2]
            ap0 = emall_sb[:].ap[0]
            ea_ap = bass.AP(
                emall_sb.tensor, emv.offset + 2, [[ap0[0], L], [S, 2], [3, NT]]
            )
            eb_ap = bass.AP(
                emall_sb.tensor, emv.offset + 3, [[ap0[0], L], [S, 2], [3, NT]]
            )
            rep_ps = ps_sm.tile([LL, 2, NT], f32, tag="smt")
            tmm(rep_ps[:], ra_ap, ea_ap, start=True, stop=False)
            tmm(rep_ps[:], rb_ap, eb_ap, start=False, stop=True)
            return rep_ps

        def stage_uexp(p, rep_ps):
            u_sb = upool.tile([LL, 2, NT], bf16)
            nc.scalar.activation(
                u_sb[:], rep_ps[:], Act.Exp, bias=bias81, scale=1.0 / SCALE_W
            )
            usbs[p] = u_sb

        def stage_g6(p):
            # T^T [(i,j), (i,q)] = G6^T @ u  (pair of triple-transfer matrices)
            t_ps = ps_sm.tile([LL, 2, NT], f32, tag="smt")
            tmm(t_ps[:], g6_ap, usbs[p][:], start=True, stop=True)
            g6ps[p] = t_ps

        def stage_coll(p):
            nc.scalar.copy(coll_sb[:, 2 * p : 2 * p + 2], g6ps[p][:])
            emps[p] = usbs[p] = g6ps[p] = None

        stage_mm(0)
        stage_cast(0)
        stage_mm(1)
        stage_cast(1)
        stage_uexp(0, stage_rep(0))
        for p in range(2, NSP):
            stage_mm(p)
            if p < NSP - 1:
                stage_cast(p)
            stage_uexp(p - 1, stage_rep(p - 1))
            stage_g6(p - 2)
            stage_coll(p - 2)
        # last pair: per-sequence tail to shorten the pipeline drain
        pL = NSP - 1
        emL = emps[pL]
        ap0 = emall_sb[:].ap[0]
        repL = [None, None]
        for i in range(2):
            bL = 2 * pL + i
            nc.vector.tensor_copy(emall_sb[:, bL : bL + 1], emL[0:L, i : i + 1])
            emv = emall_sb[:, bL : bL + 1]
            ea_ap = bass.AP(
                emall_sb.tensor, emv.offset + 2, [[ap0[0], L], [1, 1], [3, NT]]
            )
            eb_ap = bass.AP(
                emall_sb.tensor, emv.offset + 3, [[ap0[0], L], [1, 1], [3, NT]]
            )
            rep_ps = ps_sm.tile([LL, 2, NT], f32, tag="smt")
            tmm(rep_ps[:, 0], ra_ap, ea_ap, start=True, stop=False)
            tmm(rep_ps[:, 0], rb_ap, eb_ap, start=False, stop=True)
            repL[i] = rep_ps
        stage_g6(NSP - 2)
        stage_coll(NSP - 2)
        # first-half outputs overlap with the last pair's tail
        nc.sync.dma_start(t_out[:, 0:4], coll_sb[:, 0:4])
        nc.sync.dma_start(em_out[:, 0:4], emall_sb[:, 0:4])
        uL = [None, None]
        for i in range(2):
            u_sb = upool.tile([LL, 1, NT], bf16, tag="ulast")
            nc.scalar.activation(
                u_sb[:], repL[i][:, 0], Act.Exp, bias=bias81, scale=1.0 / SCALE_W
            )
            uL[i] = u_sb
        for i in range(2):
            bL = 2 * pL + i
            t_ps = ps_sm.tile([LL, 2, NT], f32, tag="smt")
            tmm(t_ps[:, 0], g6_ap, uL[i][:, 0], start=True, stop=True)
            nc.scalar.copy(coll_sb[:, bL : bL + 1], t_ps[:, 0:1])

        nc.sync.dma_start(t_out[:, 4:BPC], coll_sb[:, 4:BPC])
        nc.sync.dma_start(em_out[:, 4:BPC], emall_sb[:, 4:BPC])

    if not nc.is_finalized():
        nc.finalize()
    return nc


def _get_nc():
    if "nc" not in _CACHE:
        _CACHE["nc"] = _build_bass()
    return _CACHE["nc"]


def _host_consts(trans, bb):
    expT = np.exp(trans.astype(np.float64))                      # [9,9] f64
    r = np.arange(LL)
    c = np.arange(LL)
    k = r // L
    l = r % L
    i = c // L
    j = c % L
    gpack = np.zeros((LL, 3 * LL), dtype=ml_dtypes.bfloat16)
    # G6[(k,l), (i,j)] = expT[i,k] * expT[k,l] * expT[l,j]
    gpack[:, 0:LL] = (
        expT[i[None, :], k[:, None]]
        * expT[k[:, None], l[:, None]]
        * expT[l[:, None], j[None, :]]
    ).astype(ml_dtypes.bfloat16)
    gpack[0:L, LL : 2 * LL] = k[None, :] == np.arange(L)[:, None]
    gpack[0:L, 2 * LL : 3 * LL] = l[None, :] == np.arange(L)[:, None]
    b64 = bb.astype(np.float64)
    cf = (b64[k] + b64[l]).astype(np.float32).reshape(LL, 1)
    return expT, gpack, cf


def _numpy_reference(hs, mask, labels, W, bb, st, en, tr):
    # general fallback (only used when attention_mask is not all ones)
    em = hs.astype(np.float64) @ W.astype(np.float64) + bb.astype(np.float64)
    maskb = mask.astype(bool)
    maskf = mask.astype(np.float64)
    em_tag = np.take_along_axis(em, labels[..., None], axis=-1)[..., 0]
    num = st.astype(np.float64)[labels[:, 0]] + em_tag[:, 0]
    trs = tr.astype(np.float64)[labels[:, :-1], labels[:, 1:]]
    num = num + np.sum((trs + em_tag[:, 1:]) * maskf[:, 1:], axis=1)
    last = mask.sum(axis=1).astype(np.int64) - 1
    num = num + en.astype(np.float64)[labels[np.arange(len(labels)), last]]
    alpha = st.astype(np.float64)[None, :] + em[:, 0]
    for t in range(1, em.shape[1]):
        x = alpha[:, :, None] + tr.astype(np.float64)[None, :, :] + em[:, t][:, None, :]
        m = x.max(axis=1, keepdims=True)
        nxt = np.log(np.exp(x - m).sum(axis=1)) + m[:, 0, :]
        alpha = np.where(maskb[:, t][:, None], nxt, alpha)
    x = alpha + en.astype(np.float64)[None, :]
    m = x.max(axis=1, keepdims=True)
    denom = np.log(np.exp(x - m).sum(axis=1)) + m[:, 0]
    return np.asarray((denom - num).sum(), dtype=np.float32)


def _run_device(nc, in_maps):
    if os.environ.get("KERNEL_SIM"):
        from concourse.bass_interp import MultiCoreSim

        sim = MultiCoreSim(nc, len(in_maps))
        for t, m in enumerate(in_maps):
            for k2, v in m.items():
                sim.cores[t].tensor(k2)[:] = v
        sim.simulate()
        outs = []
        for t in range(len(in_maps)):
            outs.append(
                {
                    name: np.array(sim.cores[t].tensor(name))
                    for name in ("t_out", "em_out")
                }
            )

        class _R:
            results = outs
            exec_time_ns = None

        return _R()
    from concourse import bass_utils

    return bass_utils.run_bass_kernel_spmd(nc, in_maps, list(range(len(in_maps))))


def kernel(**inputs):
    hs = np.asarray(inputs["hidden_states"], dtype=np.float32)
    mask = np.asarray(inputs["attention_mask"])
    labels = np.asarray(inputs["labels"]).astype(np.int64)
    W = np.asarray(inputs["W"], dtype=np.float32)
    bb = np.asarray(inputs["b"], dtype=np.float32)
    st = np.asarray(inputs["start_trans"], dtype=np.float32)
    en = np.asarray(inputs["end_trans"], dtype=np.float32)
    tr = np.asarray(inputs["trans"], dtype=np.float32)

    if not np.all(mask == 1):
        return _numpy_reference(hs, mask, labels, W, bb, st, en, tr)

    expT64, gpack, cf = _host_consts(tr, bb)

    # X -> fp8 e4m3 in [B, 128, HC, S] layout (h = 128*c + p)
    xq = np.clip(hs, -224.0, 224.0).astype(ml_dtypes.float8_e4m3)   # [B, S, H]
    x8 = np.ascontiguousarray(
        xq.transpose(0, 2, 1)
        .reshape(B, NKT, 2, 128, S)
        .transpose(0, 3, 1, 4, 2)
    )                                                               # [B, 128, NKT, S, 2]
    wpad = np.zeros((H, MP), dtype=np.float32)
    wpad[:, :L] = W * SCALE_W
    w8 = np.ascontiguousarray(
        np.clip(wpad, -224.0, 224.0)
        .astype(ml_dtypes.float8_e4m3)
        .reshape(HC, 128, MP)
        .transpose(1, 0, 2)
    )                                                               # [128, HC, MP]

    nc = _get_nc()
    in_maps = []
    for k in range(NCORES):
        sl = slice(k * BPC, (k + 1) * BPC)
        in_maps.append(
            {"x8": x8[sl], "w8": w8, "Gpack": gpack, "Cf32": cf}
        )
    res = _run_device(nc, in_maps)
    _CACHE["last_results"] = res

    # ---- host combine (f64, O(B * NT * 81)) ----
    st64 = st.astype(np.float64)
    en64 = en.astype(np.float64)
    b64 = bb.astype(np.float64)
    e_en = np.exp(en64)
    e_st = np.exp(st64)
    total = 0.0
    for k in range(NCORES):
        r = res.results[k]
        em = r["em_out"].astype(np.float64) / SCALE_W       # [9, b, t] (no bias)
        Tm = (
            r["t_out"].astype(np.float64)
            .reshape(L, L, BPC, NT)
            .transpose(2, 3, 0, 1)
        )                                                   # [b, q, i, j]
        # diag factors exp(em + b) at t = 3q+4
        D = np.exp(em[:, :, 4::3] + b64[:, None, None]).transpose(1, 2, 0)
        M = Tm * D[:, :, None, :]
        logacc = np.zeros(BPC)
        while M.shape[1] > 1:
            n = M.shape[1]
            half = n // 2
            P = M[:, 0 : 2 * half : 2] @ M[:, 1 : 2 * half : 2]
            if n % 2:
                P = np.concatenate([P, M[:, 2 * half :]], axis=1)
            m = P.max(axis=(2, 3), keepdims=True)
            P /= m
            logacc += np.log(m[:, :, 0, 0]).sum(axis=1)
            M = P
        v0 = e_st[:, None] * np.exp(em[:, :, 0] + b64[:, None])     # [9, b]
        v1 = (v0.T @ expT64) * np.exp(em[:, :, 1] + b64[:, None]).T
        v = np.einsum("bj,bjk->bk", v1, M[:, 0])
        denom = np.log(v @ e_en) + logacc
        total += float(denom.sum())
        lb = labels[k * BPC : (k + 1) * BPC]                # [b, t]
        em_tag = np.take_along_axis(em.transpose(1, 2, 0), lb[:, :, None], axis=2)
        total -= float(em_tag.sum())
        total -= float(
            st64[lb[:, 0]].sum()
            + en64[lb[:, -1]].sum()
            + tr.astype(np.float64)[lb[:, :-1], lb[:, 1:]].sum()
            + bb.astype(np.float64)[lb].sum()
        )
    return np.asarray(total, dtype=np.float32)
